# revision 11
# baseline (speedup 1.0000x reference)
"""Multi-head attention (B=2, S=2048, D=2048, H=16) on 8 TRN2 NeuronCores.

Sharding: data-parallel over batch (2) x Megatron tensor-parallel over heads
(4 groups of 4 heads). Core c = 4*b + g handles batch b, heads [4g, 4g+4).
Each core computes q/k/v projections for its head slice, attention over its
4 heads, and a partial o_proj contribution; the host sums the 4 partials per
batch (the unshard step of Megatron TP) and stacks the 2 batches.

All matmuls are fp32r (TF32-like, full PE rate) with 512-wide moving
operands so LDWEIGHTS hides under the previous matmul. k, v and q all stay
resident in SBUF between projection and attention -- no DRAM spill. SBUF
is kept within budget by phase-scoped pools: B1 (wk+wv+x), then B2 (wq+x,
x re-DMAed), then the attention-phase pools.

The attention phase is software-pipelined for the in-order engine queues:
the attn@v matmul is skewed one kt-tile behind the scores matmul (so the PE
never waits on ACT's exp), each head's softmax tail (denominator colsum,
reciprocal, broadcast, normalize) is injected into the next head's kt loop,
and each chunk's o_proj groups are injected between the next chunk's heads.

Device schedule:
  B1: per 512-column chunk of x: k and v projections (8 psum accumulation
      groups of 16 N=512 matmuls each).
  B2: per chunk: q projection into the resident qT tile.
  C:  per 512-query chunk: per head {16x (scores matmul, exp on ACT,
      denominator accumulation split DVE/GPSIMD ~2:1, attn@v matmul
      accumulated in PSUM)}, tails and o_proj skewed as above.
"""

import math
import os

import numpy as np

import concourse.mybir as mybir
import concourse.tile as tile
from concourse import bacc
from concourse.bass_utils import run_bass_kernel_spmd

F32 = mybir.dt.float32
F32R = mybir.dt.float32r

B, S, D = 2, 2048, 2048
H = 16
HD = 128
G = 4            # tensor-parallel groups
HLOC = H // G    # heads per core = 4
DG = HLOC * HD   # per-core projection width = 512
P = 128
NCORES = 8

SCHUNK = 512
NSC = S // SCHUNK          # 4 chunks
DT = D // P                # 16 contraction tiles
MT = DG // P               # 4 output tiles (= heads) per projection
KT = S // P                # 16 key tiles
QC = NSC                   # 4 query chunks
IC = D // SCHUNK           # 4 o_proj output column blocks
ST_PER_CHUNK = SCHUNK // P # 4 row tiles per chunk
INV_SQRT_HD = 1.0 / math.sqrt(HD)

_cache = {}
last_run = None  # BassKernelResults of the most recent execution (for test.py)


def build(loop_reps=None):
    nc = bacc.Bacc(None, target_bir_lowering=False)

    xT_dr = nc.dram_tensor("xT", [D, S], F32R, kind="ExternalInput")
    wkT_dr = nc.dram_tensor("wkT", [D, DG], F32R, kind="ExternalInput")
    wqT_dr = nc.dram_tensor("wqT", [D, DG], F32R, kind="ExternalInput")
    wvT_dr = nc.dram_tensor("wvT", [D, DG], F32R, kind="ExternalInput")
    woT_dr = nc.dram_tensor("woT", [DG, D], F32R, kind="ExternalInput")
    out_d = nc.dram_tensor("out", [S, D], F32, kind="ExternalOutput")

    xT_view = xT_dr.rearrange("(o p) s -> p o s", p=P)
    wkT_v = wkT_dr.rearrange("(o p) m -> p o m", p=P)
    wqT_v = wqT_dr.rearrange("(o p) m -> p o m", p=P)
    wvT_v = wvT_dr.rearrange("(o p) m -> p o m", p=P)
    woT_v = woT_dr.rearrange("(o p) i -> p o i", p=P)

    import contextlib

    with tile.TileContext(nc) as tc:
        loop_cm = tc.For_i(0, loop_reps, 1) if loop_reps else contextlib.nullcontext()
        with loop_cm:
            with (
                tc.tile_pool(name="ktres", bufs=1) as ktpool,
                tc.tile_pool(name="vres", bufs=1) as vpool,
            ):
                # k^T resident: [HD part, head, key]
                kT = ktpool.tile([P, HLOC, S], F32R, tag="kT")
                # v resident: [key-in-tile part, key tile, head, hd]
                vv = vpool.tile([P, KT, HLOC, HD], F32R, tag="vv")

                # ---------- B1: k and v projections ----------
                with (
                    tc.tile_pool(name="wkv", bufs=1) as wpool,
                    tc.tile_pool(name="xt1", bufs=2) as xtpool,
                    tc.tile_pool(name="psumB1", bufs=1, space="PSUM") as psumB,
                ):
                    # first x chunk before the weights so compute starts early
                    xt0 = xtpool.tile([P, DT, SCHUNK], F32R, tag="xt")
                    for d0 in range(0, DT, 4):
                        nc.sync.dma_start(
                            xt0[:, d0:d0 + 4], xT_view[:, d0:d0 + 4, 0:SCHUNK])

                    wkT = wpool.tile([P, DT, DG], F32R, tag="wkT")
                    wvT = wpool.tile([P, DT, DG], F32R, tag="wvT")
                    for d0 in range(0, DT, 4):
                        nc.sync.dma_start(wkT[:, d0:d0 + 4], wkT_v[:, d0:d0 + 4])
                    for d0 in range(0, DT, 4):
                        nc.sync.dma_start(wvT[:, d0:d0 + 4], wvT_v[:, d0:d0 + 4])

                    for sc in range(NSC):
                        c0 = sc * SCHUNK
                        if sc == 0:
                            xt = xt0
                        else:
                            xt = xtpool.tile([P, DT, SCHUNK], F32R, tag="xt")
                            for d0 in range(0, DT, 4):
                                nc.sync.dma_start(
                                    xt[:, d0:d0 + 4],
                                    xT_view[:, d0:d0 + 4, c0:c0 + SCHUNK])

                        # k projection: out tile [head-dims 128, 512 keys]
                        for mt in range(MT):
                            ps = psumB.tile([P, SCHUNK], F32, tag="psB", bufs=4)
                            for dt in range(DT):
                                nc.tensor.matmul(
                                    ps[:], wkT[:, dt, mt * P:(mt + 1) * P],
                                    xt[:, dt, :],
                                    start=(dt == 0), stop=(dt == DT - 1))
                            nc.vector.tensor_copy(kT[:, mt, c0:c0 + SCHUNK], ps[:])

                        # v projection: out tile [keys 128, (head, hd) 512]
                        for st in range(ST_PER_CHUNK):
                            ps = psumB.tile([P, DG], F32, tag="psB", bufs=4)
                            for dt in range(DT):
                                nc.tensor.matmul(
                                    ps[:], xt[:, dt, st * P:(st + 1) * P],
                                    wvT[:, dt, :],
                                    start=(dt == 0), stop=(dt == DT - 1))
                            kt_idx = sc * ST_PER_CHUNK + st
                            nc.vector.tensor_copy(
                                vv[:, kt_idx].rearrange("p h n -> p (h n)"), ps[:])

                with tc.tile_pool(name="qtres", bufs=1) as qtpool:
                    # q^T resident: [HD part, head, query]
                    qT = qtpool.tile([P, HLOC, S], F32R, tag="qT")

                    # ---------- B2: q projection ----------
                    with (
                        tc.tile_pool(name="wq", bufs=1) as wqpool,
                        tc.tile_pool(name="xt2", bufs=2) as xtpool2,
                        tc.tile_pool(name="psumB2", bufs=1, space="PSUM") as psumB2,
                    ):
                        # x chunk 0 again before wq so the seam stall is short
                        xt20 = xtpool2.tile([P, DT, SCHUNK], F32R, tag="xt2")
                        for d0 in range(0, DT, 4):
                            nc.sync.dma_start(
                                xt20[:, d0:d0 + 4], xT_view[:, d0:d0 + 4, 0:SCHUNK])

                        wqT = wqpool.tile([P, DT, DG], F32R, tag="wqT")
                        for d0 in range(0, DT, 4):
                            nc.sync.dma_start(wqT[:, d0:d0 + 4], wqT_v[:, d0:d0 + 4])

                        for sc in range(NSC):
                            c0 = sc * SCHUNK
                            if sc == 0:
                                xt = xt20
                            else:
                                xt = xtpool2.tile([P, DT, SCHUNK], F32R, tag="xt2")
                                for d0 in range(0, DT, 4):
                                    nc.sync.dma_start(
                                        xt[:, d0:d0 + 4],
                                        xT_view[:, d0:d0 + 4, c0:c0 + SCHUNK])
                            for mt in range(MT):
                                ps = psumB2.tile([P, SCHUNK], F32, tag="psB2",
                                                 bufs=4)
                                for dt in range(DT):
                                    nc.tensor.matmul(
                                        ps[:], wqT[:, dt, mt * P:(mt + 1) * P],
                                        xt[:, dt, :],
                                        start=(dt == 0), stop=(dt == DT - 1))
                                nc.vector.tensor_copy(
                                    qT[:, mt, c0:c0 + SCHUNK], ps[:])

                    # ---------- C: attention + o_proj ----------
                    with (
                        tc.tile_pool(name="wo2", bufs=1) as wopool,
                        tc.tile_pool(name="ctx", bufs=2) as ctxpool,
                        tc.tile_pool(name="expp", bufs=6) as expool,
                        tc.tile_pool(name="accp", bufs=2) as accpool,
                        tc.tile_pool(name="asmall", bufs=2) as small,
                        tc.tile_pool(name="ostg", bufs=3) as ostg,
                        tc.tile_pool(name="pss", bufs=1, space="PSUM") as psums,
                        tc.tile_pool(name="pso", bufs=1, space="PSUM") as psumo,
                        tc.tile_pool(name="psd", bufs=1, space="PSUM") as psumd,
                        tc.tile_pool(name="psj", bufs=1, space="PSUM") as psumj,
                    ):
                        woT = wopool.tile([P, MT, D], F32R, tag="woT")
                        for j0 in range(MT):
                            nc.sync.dma_start(woT[:, j0:j0 + 1], woT_v[:, j0:j0 + 1])

                        ones_raw = small.tile([P, 1], F32, tag="ones_raw", bufs=1)
                        nc.vector.memset(ones_raw[:], 1.0)
                        ones_f = small.tile([P, 1], F32R, tag="ones_f", bufs=1)
                        nc.vector.tensor_copy(ones_f[:], ones_raw[:])

                        # pending softmax tail of the previous head:
                        # [acc, acc2, pso, ctx_slice, pssum, recip, rb]
                        pend_tail = None
                        # pending o_proj of the previous chunk: (ctx_tile, qc)
                        pend_oproj = None

                        def tail_step(t, step):
                            if step == 0:
                                t["pssum"] = psumd.tile(
                                    [1, SCHUNK], F32, tag="pssum", bufs=1,
                                    name="pssum_t")
                                nc.tensor.matmul(
                                    t["pssum"][:], ones_f[:], t["acc"][:],
                                    start=True, stop=False)
                                nc.tensor.matmul(
                                    t["pssum"][:], ones_f[:], t["acc2"][:],
                                    start=False, stop=True)
                            elif step == 1:
                                t["recip"] = small.tile(
                                    [1, SCHUNK], F32, tag="recip",
                                    name="recip_t")
                                nc.vector.reciprocal(t["recip"][:], t["pssum"][:])
                            elif step == 2:
                                t["rb"] = small.tile([P, SCHUNK], F32, tag="rb",
                                                      name="rb_t")
                                nc.gpsimd.partition_broadcast(
                                    t["rb"][:], t["recip"][:])
                            elif step == 3:
                                nc.vector.tensor_mul(
                                    t["ctx_slice"], t["pso"][:], t["rb"][:])

                        def oproj_group(octx, oqc, st):
                            stile = oqc * ST_PER_CHUNK + st
                            for ic in range(IC):
                                ps = psumj.tile([P, SCHUNK], F32,
                                                tag="opsum", bufs=2)
                                for jt in range(MT):
                                    nc.tensor.matmul(
                                        ps[:],
                                        octx[:, jt, st * P:(st + 1) * P],
                                        woT[:, jt,
                                            ic * SCHUNK:(ic + 1) * SCHUNK],
                                        start=(jt == 0), stop=(jt == MT - 1))
                                ob = ostg.tile([P, SCHUNK], F32, tag="ostage")
                                nc.vector.tensor_copy(ob[:], ps[:])
                                nc.sync.dma_start(
                                    out_d[stile * P:(stile + 1) * P,
                                          ic * SCHUNK:(ic + 1) * SCHUNK],
                                    ob[:])

                        for qc in range(QC):
                            q0 = qc * SCHUNK
                            ctx = ctxpool.tile([P, HLOC, SCHUNK], F32R, tag="ctx")
                            for h in range(HLOC):
                                acc = accpool.tile([P, SCHUNK], F32R, tag="acc")
                                acc2 = accpool.tile([P, SCHUNK], F32R, tag="acc2")
                                pso = psumo.tile([P, SCHUNK], F32, tag="pso",
                                                 bufs=2)
                                prev_exp = None
                                for kt in range(KT):
                                    pss = psums.tile([P, SCHUNK], F32, tag="pss",
                                                     bufs=3)
                                    nc.tensor.matmul(
                                        pss[:], kT[:, h, kt * P:(kt + 1) * P],
                                        qT[:, h, q0:q0 + SCHUNK],
                                        start=True, stop=True)
                                    # attn@v skewed one tile behind scores so
                                    # the in-order PE never waits on ACT's exp
                                    if kt >= 1:
                                        nc.tensor.matmul(
                                            pso[:], vv[:, kt - 1, h, :],
                                            prev_exp[:],
                                            start=(kt == 1), stop=False)
                                    expP = expool.tile([P, SCHUNK], F32R,
                                                       tag="expP")
                                    nc.scalar.activation(
                                        expP[:], pss[:],
                                        mybir.ActivationFunctionType.Exp,
                                        scale=INV_SQRT_HD)
                                    expf = expP[:].bitcast(F32)
                                    # denominator: 2 chains, DVE:GPSIMD ~ 2:1
                                    if kt == 0:
                                        nc.vector.tensor_copy(acc[:], expf)
                                    elif kt == 1:
                                        nc.gpsimd.tensor_copy(acc2[:], expf)
                                    elif kt % 3 == 1:
                                        nc.gpsimd.tensor_add(acc2[:], acc2[:], expf)
                                    else:
                                        nc.vector.tensor_add(acc[:], acc[:], expf)
                                    prev_exp = expP
                                    # previous head's softmax tail, injected
                                    # early in this head's loop
                                    if pend_tail is not None and 1 <= kt <= 4:
                                        tail_step(pend_tail, kt - 1)
                                        if kt == 4:
                                            pend_tail = None
                                nc.tensor.matmul(
                                    pso[:], vv[:, KT - 1, h, :], prev_exp[:],
                                    start=False, stop=True)
                                pend_tail = {
                                    "acc": acc, "acc2": acc2, "pso": pso,
                                    "ctx_slice": ctx[:, h, :],
                                }
                                # previous chunk's o_proj, one row-tile per head
                                if pend_oproj is not None:
                                    oproj_group(pend_oproj[0], pend_oproj[1], h)
                            if pend_oproj is not None:
                                pend_oproj = None
                            pend_oproj = (ctx, qc)

                        # drain the last head's tail and the last chunk's o_proj
                        for step in range(4):
                            tail_step(pend_tail, step)
                        for st in range(ST_PER_CHUNK):
                            oproj_group(pend_oproj[0], pend_oproj[1], st)

    nc.finalize()
    return nc

_build = build


def _round_f32r(a):
    """Round fp32 to fp32r bit patterns (round-to-nearest-even to 12 explicit
    mantissa bits, TF32-like) -- matches the hardware's own rounding."""
    u = np.ascontiguousarray(a, dtype=np.float32).view(np.uint32)
    keep = np.uint32(0xFFFFF000)
    half = np.uint32(0x7FF)
    lsb = (u >> np.uint32(12)) & np.uint32(1)
    return ((u + half + lsb) & keep).view(np.float32)


def kernel(hidden_states, wq, wk, wv, wo):
    global last_run
    if "nc" not in _cache:
        _cache["nc"] = build()
    nc = _cache["nc"]

    hidden_states = np.asarray(hidden_states, dtype=np.float32)
    wq = np.asarray(wq, dtype=np.float32)
    wk = np.asarray(wk, dtype=np.float32)
    wv = np.asarray(wv, dtype=np.float32)
    wo = np.asarray(wo, dtype=np.float32)

    xT = [_round_f32r(hidden_states[b].T) for b in range(B)]
    in_maps = []
    for c in range(NCORES):
        b, g = divmod(c, G)
        sl = slice(g * DG, (g + 1) * DG)
        in_maps.append({
            "xT": xT[b],
            "wqT": _round_f32r(wq[sl, :].T),
            "wkT": _round_f32r(wk[sl, :].T),
            "wvT": _round_f32r(wv[sl, :].T),
            "woT": _round_f32r(wo[:, sl].T),
        })

    trace = os.environ.get("BASSKERNEL_TRACE", "0") == "1"
    last_run = run_bass_kernel_spmd(
        nc, in_maps, core_ids=list(range(NCORES)), trace=trace)

    out = np.empty((B, S, D), dtype=np.float32)
    for b in range(B):
        acc = None
        for g in range(G):
            part = last_run.results[b * G + g]["out"]
            acc = part.copy() if acc is None else acc + part
        out[b] = acc
    return out
